# revision 6
# baseline (speedup 1.0000x reference)
"""BrightnessLoss Trainium2 kernel (raw Bass, 8-core data parallel).

reference:
    V(x)   = max_c(clip(x, 0, 1))        over channel dim (RGB)
    result = mean(|V(pred) - V(target)|) over (N, H, W)

Identities used on device:
    clip(max(r,g,b),0,1) == max_c(clip(x,0,1))          (clip is monotone)
    W := relu(1 - relu(m)) == 1 - clip(m, 0, 1)
    |Vp - Vt| == |Wp - Wt|  and  sum|a-b| == sum max(a,b) - sum min(a,b)

Per core (8 cores, 4 images each), per image n:
    SP   2x dma  [128, 3*2048] f32 (3 MB each)  pred / target planes
    DVE  m1 = max(R,G); u = (m1 max 0) max B    (fused relu, x2 sides)
    ACT  W = Relu(-u + 1)                       (x2 sides)
    DVE  tensor_tensor_reduce (max,add) / (min,add) -> acc[:, 2n], acc[:, 2n+1]
    SP   dma acc [128, 2*n_img] -> partials

Software pipeline: images double-buffered (parity slots); DVE stream is
[loads/maxes/STTs img n ; TTRs img n-1] so DVE never idles on ACT.
Host sums per-core partials in float64: sum|Vp-Vt| = sum(even) - sum(odd).
"""

import numpy as np

N_CORES = 8
N_IMG = 4  # 32 / 8
C = 3
P = 128
F = 2048  # 512*512 / 128
N_PIX = 32 * 512 * 512


def _build_program(n_img=N_IMG, f=F):
    from contextlib import ExitStack

    import concourse.bass as bass
    import concourse.mybir as mybir

    fp32 = mybir.dt.float32
    Alu = mybir.AluOpType
    Act = mybir.ActivationFunctionType

    # detect_race_conditions=False: the raw-mode CoreSim race detector can't
    # see same-engine program-order (DVE m1 -> STT RAW); hardware engines
    # execute in order (DVE drains its pipe after every op).
    nc = bass.Bass(
        "TRN2", target_bir_lowering=False, debug=False, detect_race_conditions=False
    )
    pred = nc.dram_tensor("pred", [n_img, C, P, f], fp32, kind="ExternalInput").ap()
    targ = nc.dram_tensor("target", [n_img, C, P, f], fp32, kind="ExternalInput").ap()
    out = nc.dram_tensor("partials", [P, 2 * n_img], fp32, kind="ExternalOutput").ap()

    with ExitStack() as ctx:
        sb = lambda name, shape: ctx.enter_context(nc.sbuf_tensor(name, shape, fp32))
        sem = lambda name: ctx.enter_context(nc.semaphore(name))

        inb = [
            [sb(f"in{sl}{s}", [P, C * f]) for s in range(2)] for sl in range(2)
        ]  # [slot][side]
        ub = [[sb(f"u{sl}{s}", [P, f]) for s in range(2)] for sl in range(2)]
        wb = [[sb(f"w{sl}{s}", [P, f]) for s in range(2)] for sl in range(2)]
        m1 = sb("m1", [P, f])
        dummy = sb("ttr_scratch", [P, f])  # TTR main output, discarded
        acc = sb("acc", [P, 2 * n_img])

        in_sem = [sem("in0"), sem("in1")]
        u_sem = sem("u")
        act_sem = sem("act")
        done_sem = sem("done")
        out_sem = sem("outd")

        block = ctx.enter_context(nc.Block())

        @block.sync
        def _(sync):
            for n in range(n_img):
                if n >= 2:
                    # WAR on inb[n%2]: image n-2's STTs (last input readers)
                    sync.wait_ge(u_sem, 2 * (n - 1))
                for side_ap in (pred, targ):
                    sync.dma_start(
                        out=inb[n % 2][0 if side_ap is pred else 1][:].rearrange(
                            "p (c f) -> p c f", c=C
                        ),
                        in_=side_ap[n].rearrange("c p f -> p c f"),
                    ).then_inc(in_sem[n % 2], 16)
            sync.wait_ge(done_sem, 1)
            sync.dma_start(out=out[:], in_=acc[:]).then_inc(out_sem, 16)
            sync.wait_ge(out_sem, 16)

        @block.vector
        def _(vector):
            def ttrs(n):
                vector.wait_ge(act_sem, 2 * (n + 1))
                for k, op in enumerate((Alu.max, Alu.min)):
                    # out = (Wp bypass 0) op Wt ; accum = sum(out)
                    inst = vector.scalar_tensor_tensor(
                        dummy[:],
                        wb[n % 2][0][:],
                        0.0,
                        wb[n % 2][1][:],
                        op0=Alu.bypass,
                        op1=op,
                        accum_out=acc[:, 2 * n + k : 2 * n + k + 1],
                    )
                    if n == n_img - 1 and k == 1:
                        inst.then_inc(done_sem, 1)

            for n in range(n_img):
                vector.wait_ge(in_sem[n % 2], 32 * (n // 2 + 1))
                for s in range(2):
                    t = inb[n % 2][s]
                    vector.tensor_max(m1[:], t[:, 0:f], t[:, f : 2 * f])
                    vector.scalar_tensor_tensor(
                        ub[n % 2][s][:],
                        m1[:],
                        0.0,
                        t[:, 2 * f : 3 * f],
                        op0=Alu.max,
                        op1=Alu.max,
                    ).then_inc(u_sem, 1)
                if n > 0:
                    ttrs(n - 1)
            ttrs(n_img - 1)

        @block.scalar
        def _(scalar):
            for n in range(n_img):
                for s in range(2):
                    scalar.wait_ge(u_sem, 2 * n + s + 1)
                    scalar.activation(
                        wb[n % 2][s][:],
                        ub[n % 2][s][:],
                        Act.Relu,
                        bias=1.0,
                        scale=-1.0,
                    ).then_inc(act_sem, 1)

    return nc


_program = None


def _get_program():
    global _program
    if _program is None:
        _program = _build_program()
    return _program


def _finish(partials_list):
    """partials_list: per-core [P, 2*n_img] f32; even cols = sum max(Wp,Wt),
    odd cols = sum min(Wp,Wt). sum|Vp-Vt| = sum(even) - sum(odd)."""
    total = np.float64(0.0)
    for p in partials_list:
        p = p.astype(np.float64)
        total += p[:, 0::2].sum() - p[:, 1::2].sum()
    return np.array(total / N_PIX, dtype=np.float32)


def kernel(pred: np.ndarray, target: np.ndarray) -> np.ndarray:
    from concourse.bass_utils import run_bass_kernel_spmd

    nc = _get_program()
    pred = np.ascontiguousarray(pred, dtype=np.float32).reshape(
        N_CORES, N_IMG, C, P, F
    )
    target = np.ascontiguousarray(target, dtype=np.float32).reshape(
        N_CORES, N_IMG, C, P, F
    )
    in_maps = [{"pred": pred[i], "target": target[i]} for i in range(N_CORES)]
    res = run_bass_kernel_spmd(nc, in_maps, list(range(N_CORES)))
    return _finish([r["partials"] for r in res.results])
